# revision 1
# baseline (speedup 1.0000x reference)
"""Trainium2 Bass kernel for nn_Depth_CA (depth-coded-aperture Wiener pipeline).

Strategy
--------
Every fft/ifft+shift combo in the reference is a constant 256x256 complex
matrix sandwich Y = A @ X @ A.T computed on the PE array as two matmul
groups with the DATA stationary and host-precomputed constants as 512-wide
moving operands (PSUM accumulation implements the complex arithmetic).

On top of the baseline scheme, three algebraic cuts:
  * Gc == conj(Fc)/N, so psf_ifr = conj(psffr)/N^2 -- the Gc psf Gc
    sandwich is removed; the Wiener kernel K is built directly from psffr.
  * The blur and Wiener inverse transforms are real fields per depth, so
    the two depths owned by a core are PAIRED as Re/Im of one complex
    sandwich: W = Gc (X (P1 + i P2)) Gc gives both depths at once.
    Blur kernel P12 = pf_d0 + i pf_d1; Wiener kernel Q12 = Kp_d0 - i Kp_d1
    consumed via conj(Q12)*resfr (the conj is folded into the cmul).
  * blur = img (*) psf is a convolution of non-negative fields, so the
    reference's abs() is an identity and is dropped; the final global
    max-normalisation cancels mid-pipeline scaling and is done on host.

Long-lived complex fields use a [Re(rb0)|Re(rb1)|Im(rb0)|Im(rb1)] packing
so complex multiplies run as 6 [128,512] elementwise ops (2 on GpSimd).

Sharding: depths padded 15->16, 2 per core across 8 cores; per-batch
AllReduce(add) for the depth-summed `result` overlapped with blur compute.
"""
import os
import sys

for _p in ("/opt/trn_rl_repo", os.path.expanduser("~/.axon_site/_ro/trn_rl_repo")):
    if os.path.isdir(_p) and _p not in sys.path:
        sys.path.insert(0, _p)

import numpy as np

N = 256
ND, NB, B = 15, 3, 4
NDP = 16               # padded depth count
NCORES = 8
DPC = NDP // NCORES    # depths per core = 2

# ---------------------------------------------------------------- host constants
def _host_constants():
    ZI, Z0, RADII, PX = 0.05, 2.5, 0.002, 6.22e-6
    F_ = 1.0 / (1.0 / ZI + 1.0 / Z0)
    L_SEN = PX * N
    L_LEN = 2 * RADII * 2
    LAMB = np.array([460.0, 550.0, 640.0]) * 1e-9

    def deta(l_um):
        l = np.asarray(l_um, dtype=np.float64)
        return (1.5375 + 0.00829045 * l**-2 - 0.000211046 * l**-4) - 1.0

    R_ = F_ * deta(5.5e-7 * 1e6)
    FLMB = R_ / deta(LAMB * 1e6)
    ZS = np.sort(-3 * np.log(np.linspace(0.9, 11, ND)) + 8)
    DU = L_LEN / N
    u = np.arange(-L_LEN / 2, L_LEN / 2, DU)
    X_, Y_ = np.meshgrid(u, u)
    XY = X_ * X_ + Y_ * Y_
    RAD = (np.sqrt(XY) <= RADII).astype(np.float64)
    fx1 = np.fft.fftshift(np.arange(-1 / (2 * DU), 1 / (2 * DU), 1 / L_LEN))
    FX1, FY1 = np.meshgrid(fx1, fx1)
    FF = FX1 * FX1 + FY1 * FY1

    K_ = 2 * np.pi / LAMB
    COEF = (-K_ / (2 * FLMB[0]))[None, :] + K_[None, :] / (2 * ZS[:, None]) \
        + (np.pi * (L_LEN - L_SEN) / (LAMB * ZI * L_LEN))[None, :]
    PHASE1 = (COEF[:, :, None, None] * XY[None, None]).astype(np.float32)
    PHASE2 = ((np.pi * LAMB * ZI * L_LEN / L_SEN)[None, :, None, None]
              * FF[None, None]).astype(np.float32)

    W1 = RAD[None, None] * np.exp(1j * PHASE1.astype(np.float64))    # (15,3,N,N)
    W2 = np.exp(-1j * PHASE2.astype(np.float64)[0])                  # (3,N,N)

    j = np.arange(N)
    F = np.exp(-2j * np.pi * np.outer(j, j) / N)
    G = np.conj(F) / N
    P = np.zeros((N, N))
    P[j, (j + N // 2) % N] = 1.0
    A1 = F @ P
    A2 = P @ G
    Fc = P @ F @ P
    Gc = P @ G @ P
    return W1, W2, (A1, A2, Fc, Gc)


def _pack_field_B(X):
    """complex (N,N) -> float32 [128, 1024] = [Re(rb0)|Re(rb1)|Im(rb0)|Im(rb1)]."""
    out = np.empty((128, 1024), np.float32)
    for k in range(2):
        out[:, k * 256:(k + 1) * 256] = X.real[k * 128:(k + 1) * 128, :]
        out[:, 512 + k * 256:512 + (k + 1) * 256] = X.imag[k * 128:(k + 1) * 128, :]
    return out


def _pack_moving(A):
    """constant A -> float32 [2 variants, 2 k-chunks, 128, 512] moving ops."""
    AT = A.T.copy()
    out = np.empty((2, 2, 128, 512), np.float32)
    for k in range(2):
        r = AT.real[k * 128:(k + 1) * 128, :]
        i = AT.imag[k * 128:(k + 1) * 128, :]
        out[0, k, :, 0:256] = r
        out[0, k, :, 256:512] = i
        out[1, k, :, 0:256] = -i
        out[1, k, :, 256:512] = r
    return out


_CONST_CACHE = {}


def _get_device_arrays():
    """Host constants packed into the device DMA layouts."""
    if "dev" not in _CONST_CACHE:
        W1, W2, mats = _host_constants()
        # moving constants [128, 8192]: col = ((a*2+v)*2+k)*512 + n
        movA = np.concatenate(
            [_pack_moving(A).reshape(4, 128, 512).transpose(1, 0, 2).reshape(128, 2048)
             for A in mats], axis=1)
        # w2 [128, 3072]: col = c*1024 + layout-B
        w2p = np.concatenate([_pack_field_B(W2[c]) for c in range(NB)], axis=1)
        # w1 table [48, 128, 1024] layout-B, d-major over padded depths
        w1rows = []
        for d in range(NDP):
            dd = d if d < ND else 0
            for c in range(NB):
                w1rows.append(_pack_field_B(W1[dd, c]))
        w1all = np.stack(w1rows)
        R = np.kron(np.eye(16), np.ones((1, 16))).astype(np.float32)
        _CONST_CACHE["dev"] = (np.ascontiguousarray(movA), np.ascontiguousarray(w2p),
                               np.ascontiguousarray(w1all), R)
    return _CONST_CACHE["dev"]


# ---------------------------------------------------------------- device program
_REPS = int(os.environ.get("BASS_KERNEL_REPS", "1"))

A1_I, A2_I, FC_I, GC_I = 0, 1, 2, 3


def _build_program():
    host_arrays = _get_device_arrays()
    reps = _REPS
    import concourse.bass as bass
    import concourse.bass_isa as bass_isa
    import concourse.bacc as bacc
    import concourse.mybir as mybir
    import concourse.tile as tile

    dt = mybir.dt
    ALU = mybir.AluOpType
    ACTF = mybir.ActivationFunctionType

    movA_h, w2_h, w1all_h, R_h = host_arrays

    nc = bacc.Bacc("TRN2", target_bir_lowering=False, debug=False,
                   num_devices=NCORES)

    def inline(data, name, f32r=False):
        h = nc.inline_tensor(np.ascontiguousarray(data), name=name)
        if f32r:
            mls = nc.lookup_mls(h)
            mls.dtype = dt.float32r
            h = bass.DRamTensorHandle(name, list(data.shape), dt.float32r)
        return h.ap()

    movA_d = inline(movA_h, "mova", f32r=True)                 # [128, 8192]
    w2_d = inline(w2_h, "w2")                                  # [128, 3072]
    w1all_d = inline(w1all_h, "w1all")                         # [48, 128, 1024]
    r_d = inline(R_h, "rmat")                                  # [16, 256]
    onesc_d = inline(np.ones((128, 1), np.float32), "onesc", f32r=True)

    img_d = nc.dram_tensor("imgf", [128, 6144], dt.float32r, kind="ExternalInput").ap()
    map_d = nc.dram_tensor("mapf", [B, 128, DPC * 512], dt.float32, kind="ExternalInput").ap()
    ht_d = nc.dram_tensor("ht", [16, 16], dt.float32, kind="ExternalInput").ap()
    par_d = nc.dram_tensor("param", [1, 1], dt.float32, kind="ExternalInput").ap()
    out_d = nc.dram_tensor("out_recov", [NB, B, 2, 128, 512], dt.float32, kind="ExternalOutput").ap()

    with tile.TileContext(nc) as tc:
        with (
            tc.tile_pool(name="res", bufs=1) as res,
            tc.tile_pool(name="wk", bufs=2) as wk,
            tc.tile_pool(name="ps", bufs=4, space="PSUM") as ps,
            tc.tile_pool(name="dram", bufs=1, space="DRAM") as dram,
        ):
            # ---------------- resident constants (single DMAs)
            movall = res.tile([128, 8192], dt.float32r, tag="movall", name="movall")
            for _a in (FC_I, A1_I, A2_I, GC_I):
                nc.sync.dma_start(movall[:, _a * 2048:(_a + 1) * 2048],
                                  movA_d[:, _a * 2048:(_a + 1) * 2048])

            def mov(a, v, k):
                o = ((a * 2 + v) * 2 + k) * 512
                return movall[:, o:o + 512]

            w2all = res.tile([128, 3072], dt.float32, tag="w2all", name="w2all")
            nc.sync.dma_start(w2all[:], w2_d[:])

            par1 = res.tile([1, 1], dt.float32, tag="par1", name="par1")
            nc.scalar.dma_start(par1[:], par_d[:])
            par128 = res.tile([128, 1], dt.float32, tag="par128", name="par128")
            nc.gpsimd.partition_broadcast(par128[:], par1[:])
            # parameter * N^2 (the K denominator is scaled by N^2 = 65536)
            parn2 = res.tile([128, 1], dt.float32, tag="parn2", name="parn2")
            nc.vector.tensor_scalar_mul(parn2[:], par128[:], 65536.0)
            # ones vectors for matmul-based partition sum / broadcast
            ones_col = res.tile([128, 1], dt.float32r, tag="ones_col", name="ones_col")
            nc.scalar.dma_start(ones_col[:], onesc_d[:])
            ones_row = res.tile([1, 128], dt.float32, tag="ones_row", name="ones_row")
            nc.vector.memset(ones_row[:], 1.0)

            # ---------------- CA = R^T @ (H @ R)  (plain fp32), both row blocks
            ht_t = res.tile([16, 16], dt.float32, tag="ht_t", name="ht_t")
            r_t = res.tile([16, 256], dt.float32, tag="r_t", name="r_t")
            nc.scalar.dma_start(ht_t[:], ht_d[:])
            nc.scalar.dma_start(r_t[:], r_d[:])
            ca_mid_ps = ps.tile([16, 256], dt.float32, tag="psB", bufs=4, name="ca_mid_ps")
            nc.tensor.matmul(ca_mid_ps[:], ht_t[:], r_t[:], start=True, stop=True)
            ca_mid = res.tile([16, 256], dt.float32, tag="ca_mid", name="ca_mid")
            nc.vector.tensor_copy(ca_mid[:], ca_mid_ps[:])
            ca_all = res.tile([128, 512], dt.float32, tag="ca_all", name="ca_all")
            for mb in range(2):
                ca_ps = ps.tile([128, 256], dt.float32, tag="psB", bufs=4, name=f"ca_ps{mb}")
                nc.tensor.matmul(ca_ps[:], r_t[:, mb * 128:(mb + 1) * 128],
                                 ca_mid[:], start=True, stop=True)
                nc.vector.tensor_copy(ca_all[:, mb * 256:(mb + 1) * 256], ca_ps[:])

            # ---------------- stationary accessors
            def statA(tiles):
                """drained MM1 pair: per-k [128,512] = [Re|Im]."""
                re = lambda k, mb: tiles[k][:, mb * 128:(mb + 1) * 128]
                im = lambda k, mb: tiles[k][:, 256 + mb * 128:256 + (mb + 1) * 128]
                return re, im

            def statB(t):
                """layout-B field [128,1024] = [Re0|Re1|Im0|Im1]."""
                re = lambda k, mb: t[:, k * 256 + mb * 128:k * 256 + (mb + 1) * 128]
                im = lambda k, mb: t[:, 512 + k * 256 + mb * 128:512 + k * 256 + (mb + 1) * 128]
                return re, im

            def statR(t):
                """real field [128,512] = [rb0|rb1]."""
                re = lambda k, mb: t[:, k * 256 + mb * 128:k * 256 + (mb + 1) * 128]
                return re, None

            MM1_NAMES = ("s1a", "s1c", "pfa", "ifa", "bla", "rfa", "wna")

            def mm_group(stat, a_idx, name):
                """PSUM[mb][128,512] = sandwich-half vs constants a_idx."""
                s_re, s_im = stat
                ptag = "psA" if name in MM1_NAMES else "psB"
                psums = []
                for mb in range(2):
                    acc = ps.tile([128, 512], dt.float32, tag=ptag, bufs=4, name=f"{name}_ps{mb}")
                    mms = []
                    for k in range(2):
                        mms.append((s_re(k, mb), mov(a_idx, 0, k)))
                        if s_im is not None:
                            mms.append((s_im(k, mb), mov(a_idx, 1, k)))
                    for i, (lhsT, rhs) in enumerate(mms):
                        nc.tensor.matmul(acc[:], lhsT, rhs,
                                         start=(i == 0), stop=(i == len(mms) - 1))
                    psums.append(acc)
                return psums

            def drain_f32r(psums, name):
                dtag, dbufs = ("drB", 6) if name in ("blu", "wnu") else ("drA", 8)
                out = [wk.tile([128, 512], dt.float32r, tag=dtag, bufs=dbufs, name=f"{name}{mb}")
                       for mb in range(2)]
                nc.scalar.copy(out[0][:], psums[0][:])
                nc.vector.tensor_copy(out[1][:], psums[1][:])
                return out

            def drain_B(psums, btile, split=False):
                """drain PSUM pair into a layout-B field tile (4 copies)."""
                for rb in range(2):
                    nc.scalar.copy(btile[:, rb * 256:(rb + 1) * 256], psums[rb][:, 0:256])
                    eng = nc.vector.tensor_copy if split else (
                        lambda o, i: nc.scalar.copy(o, i))
                    eng(btile[:, 512 + rb * 256:512 + (rb + 1) * 256],
                        psums[rb][:, 256:512])

            def cmulB(out, x, y, conj_x=False, gp=True):
                """layout-B complex mult out = x*y (or conj(x)*y); 6 ops.
                gp=True offloads 2 to GpSimd (only safe for code emitted
                before the next collective on the gpsimd queue)."""
                eng2 = nc.gpsimd if gp else nc.vector
                xr, xi = x[:, 0:512], x[:, 512:1024]
                yr, yi = y[:, 0:512], y[:, 512:1024]
                t1 = wk.tile([128, 512], dt.float32, tag="cmw", bufs=7, name="cmt1")
                t2 = wk.tile([128, 512], dt.float32, tag="cmw", bufs=7, name="cmt2")
                t3 = wk.tile([128, 512], dt.float32, tag="cmw", bufs=7, name="cmt3")
                t4 = wk.tile([128, 512], dt.float32, tag="cmw", bufs=7, name="cmt4")
                nc.vector.tensor_tensor(t1[:], xr, yr, op=ALU.mult)
                eng2.tensor_tensor(t2[:], xi, yi, op=ALU.mult)
                nc.vector.tensor_tensor(t3[:], xr, yi, op=ALU.mult)
                eng2.tensor_tensor(t4[:], xi, yr, op=ALU.mult)
                if conj_x:
                    nc.vector.tensor_tensor(out[:, 0:512], t1[:], t2[:], op=ALU.add)
                    nc.vector.tensor_tensor(out[:, 512:1024], t3[:], t4[:], op=ALU.subtract)
                else:
                    nc.vector.tensor_tensor(out[:, 0:512], t1[:], t2[:], op=ALU.subtract)
                    nc.vector.tensor_tensor(out[:, 512:1024], t3[:], t4[:], op=ALU.add)

            # ---------------- resident per-core fields
            p12 = [res.tile([128, 1024], dt.float32, tag=f"p12_{c}", name=f"p12_{c}")
                   for c in range(NB)]
            q12 = [res.tile([128, 1024], dt.float32, tag=f"q12_{c}", name=f"q12_{c}")
                   for c in range(NB)]
            imgft_dr = dram.tile([B * NB, 128, 1024], dt.float32, name="imgft_dr")

            pid6 = nc.scalar.partition_id() * (DPC * NB)

            def emit_imgft(f):
                imS = wk.tile([128, 512], dt.float32r, tag="imS", name="imS")
                nc.scalar.dma_start(imS[:], img_d[:, f * 512:(f + 1) * 512])
                iu1 = drain_f32r(mm_group(statR(imS), FC_I, "ifa"), "ifu")
                ip2 = mm_group(statA(iu1), FC_I, "ifb")
                imo = wk.tile([128, 1024], dt.float32, tag="cfld", bufs=3, name="imo")
                drain_B(ip2, imo)
                nc.scalar.dma_start(imgft_dr[f], imo[:])

            for _rep in range(reps):
                cc_in = [dram.tile([B, 128, 512], dt.float32, name=f"cc_in{c}_r{_rep}")
                         for c in range(NB)]
                cc_out = [dram.tile([B, 128, 512], dt.float32, name=f"cc_out{c}_r{_rep}",
                                    addr_space="Shared") for c in range(NB)]

                def stage1_unit(dl, c):
                    # imgft fields of band c, 2 per depth unit, interleaved
                    # into the psf chain's dependency gaps
                    fA, fB = (c, NB + c) if dl == 0 else (2 * NB + c, 3 * NB + c)
                    u = dl * NB + c
                    w1t = wk.tile([128, 1024], dt.float32, tag="w1t", name="w1t")
                    nc.scalar.dma_start(w1t[:], w1all_d[bass.ds(pid6 + u, 1)])
                    ph = wk.tile([128, 1024], dt.float32r, tag="ph", name="ph")
                    nc.vector.tensor_tensor(ph[:, 0:512], w1t[:, 0:512], ca_all[:], op=ALU.mult)
                    nc.vector.tensor_tensor(ph[:, 512:1024], w1t[:, 512:1024], ca_all[:], op=ALU.mult)
                    u1 = drain_f32r(mm_group(statB(ph), A1_I, "s1a"), "s1u1")
                    ps2 = mm_group(statA(u1), A1_I, "s1b")
                    emit_imgft(fA)
                    # vu2 = ps2 * w2  (drain to SBUF layout-B, then wide cmul)
                    s2B = wk.tile([128, 1024], dt.float32, tag="cfld", bufs=3, name="s2B")
                    drain_B(ps2, s2B)
                    vu2 = wk.tile([128, 1024], dt.float32r, tag="cprod", bufs=3, name="vu2")
                    cmulB(vu2, s2B, w2all[:, c * 1024:(c + 1) * 1024], gp=False)
                    u3 = drain_f32r(mm_group(statB(vu2), A2_I, "s1c"), "s1u3")
                    ps4 = mm_group(statA(u3), A2_I, "s1d")
                    emit_imgft(fB)
                    # psf = |vu3|^2 normalized (real field, rb-packed [128,512])
                    psfu = wk.tile([128, 512], dt.float32r, tag="psfu", name="psfu")
                    for rb in range(2):
                        t1 = wk.tile([128, 256], dt.float32, tag="cms", bufs=12, name="sq1")
                        t2 = wk.tile([128, 256], dt.float32, tag="cms", bufs=12, name="sq2")
                        nc.scalar.activation(t1[:], ps4[rb][:, 0:256], ACTF.Square)
                        nc.scalar.activation(t2[:], ps4[rb][:, 256:512], ACTF.Square)
                        nc.vector.tensor_tensor(psfu[:, rb * 256:(rb + 1) * 256],
                                                t1[:], t2[:], op=ALU.add)
                    # partition sum via PE (ones^T @ psfu), then reciprocal +
                    # PE broadcast back to [128,1] -- keeps gpsimd queue free
                    srow_ps = ps.tile([1, 512], dt.float32, tag="psB", bufs=4, name="srow_ps")
                    nc.tensor.matmul(srow_ps[:], ones_col[:], psfu[:], start=True, stop=True)
                    tot1 = wk.tile([1, 1], dt.float32, tag="tot1", name="tot1")
                    nc.vector.tensor_reduce(tot1[:], srow_ps[:], axis=mybir.AxisListType.X, op=ALU.add)
                    inv1 = wk.tile([1, 1], dt.float32, tag="inv1", name="inv1")
                    nc.vector.reciprocal(inv1[:], tot1[:])
                    binv_ps = ps.tile([128, 1], dt.float32, tag="psB", bufs=4, name="binv_ps")
                    nc.tensor.matmul(binv_ps[:], ones_row[:], inv1[:], start=True, stop=True)
                    inv128 = wk.tile([128, 1], dt.float32, tag="inv128", name="inv128")
                    nc.vector.tensor_copy(inv128[:], binv_ps[:])
                    psft = wk.tile([128, 512], dt.float32r, tag="psft", name="psft")
                    nc.vector.tensor_scalar_mul(psft[:], psfu[:], inv128[:])
                    # psffr = Fc psf Fc
                    pu1 = drain_f32r(mm_group(statR(psft), FC_I, "pfa"), "pfu")
                    pp2 = mm_group(statA(pu1), FC_I, "pfb")
                    # P12[c] = pf(dl=0) + i pf(dl=1); pf drained to SBUF layout-B
                    if dl == 0:
                        pfB = p12[c]
                        drain_B(pp2, pfB)
                    else:
                        pfB = wk.tile([128, 1024], dt.float32, tag="cfld", bufs=3, name="pfB")
                        drain_B(pp2, pfB)
                        nc.vector.tensor_tensor(p12[c][:, 0:512], p12[c][:, 0:512],
                                                pfB[:, 512:1024], op=ALU.subtract)
                        nc.vector.tensor_tensor(p12[c][:, 512:1024], p12[c][:, 512:1024],
                                                pfB[:, 0:512], op=ALU.add)
                    # Kp = pf / (N^2 (|pf|^2 + param)); Q12[c] = Kp(0) - i Kp(1)
                    for rb in range(2):
                        t1 = wk.tile([128, 256], dt.float32, tag="cms", bufs=12, name="ab1")
                        t2 = wk.tile([128, 256], dt.float32, tag="cms", bufs=12, name="ab2")
                        nc.scalar.activation(t1[:], pp2[rb][:, 0:256], ACTF.Square, scale=256.0)
                        nc.scalar.activation(t2[:], pp2[rb][:, 256:512], ACTF.Square, scale=256.0)
                        den = wk.tile([128, 256], dt.float32, tag="cms", bufs=12, name="den")
                        nc.vector.scalar_tensor_tensor(den[:], t1[:], parn2[:], t2[:],
                                                       op0=ALU.add, op1=ALU.add)
                        invp = wk.tile([128, 256], dt.float32, tag="cms", bufs=12, name="invp")
                        nc.vector.reciprocal(invp[:], den[:])
                        if dl == 0:
                            nc.vector.tensor_tensor(q12[c][:, rb * 256:(rb + 1) * 256],
                                                    pfB[:, rb * 256:(rb + 1) * 256],
                                                    invp[:], op=ALU.mult)
                            nc.vector.tensor_tensor(q12[c][:, 512 + rb * 256:512 + (rb + 1) * 256],
                                                    pfB[:, 512 + rb * 256:512 + (rb + 1) * 256],
                                                    invp[:], op=ALU.mult)
                        else:
                            kre = wk.tile([128, 256], dt.float32, tag="cms", bufs=12, name="kre")
                            kim = wk.tile([128, 256], dt.float32, tag="cms", bufs=12, name="kim")
                            nc.vector.tensor_tensor(kre[:], pfB[:, rb * 256:(rb + 1) * 256],
                                                    invp[:], op=ALU.mult)
                            nc.vector.tensor_tensor(kim[:], pfB[:, 512 + rb * 256:512 + (rb + 1) * 256],
                                                    invp[:], op=ALU.mult)
                            nc.vector.tensor_tensor(q12[c][:, rb * 256:(rb + 1) * 256],
                                                    q12[c][:, rb * 256:(rb + 1) * 256],
                                                    kim[:], op=ALU.add)
                            nc.vector.tensor_tensor(q12[c][:, 512 + rb * 256:512 + (rb + 1) * 256],
                                                    q12[c][:, 512 + rb * 256:512 + (rb + 1) * 256],
                                                    kre[:], op=ALU.subtract)

                def blur_unit(b, c):
                    mapt = wk.tile([128, DPC * 512], dt.float32, tag="mapt", bufs=2, name="mapt")
                    nc.gpsimd.dma_start(mapt[:], map_d[b])
                    f = b * NB + c
                    imf = wk.tile([128, 1024], dt.float32, tag="cfld", bufs=3, name="imf")
                    nc.scalar.dma_start(imf[:], imgft_dr[f])
                    bw = wk.tile([128, 1024], dt.float32r, tag="cprod", bufs=3, name="bw")
                    cmulB(bw, imf, p12[c])
                    bu1 = drain_f32r(mm_group(statB(bw), GC_I, "bla"), "blu")
                    bp2 = mm_group(statA(bu1), GC_I, "blb")
                    # racc[rb] = sum_d map_d * W_d  (W re = d0, im = d1)
                    racc = wk.tile([128, 512], dt.float32, tag="racc", name="racc")
                    for rb in range(2):
                        t = wk.tile([128, 512], dt.float32, tag="cmw", bufs=7, name="bt")
                        nc.vector.tensor_tensor(t[:], bp2[rb][:],
                                                mapt[:, rb * 512:(rb + 1) * 512], op=ALU.mult)
                        nc.vector.tensor_tensor(racc[:, rb * 256:(rb + 1) * 256],
                                                t[:, 0:256], t[:, 256:512], op=ALU.add)
                    nc.sync.dma_start(cc_in[c][b], racc[:])

                def wiener_unit(b, c):
                        rres = wk.tile([128, 512], dt.float32, tag="rres", name="rres")
                        nc.scalar.dma_start(rres[:], cc_out[c][b])
                        res_t = wk.tile([128, 512], dt.float32r, tag="res_t", name="res_t")
                        nc.vector.tensor_copy(res_t[:], rres[:])
                        ru1 = drain_f32r(mm_group(statR(res_t), FC_I, "rfa"), "rfu")
                        rp2 = mm_group(statA(ru1), FC_I, "rfb")
                        resfr = wk.tile([128, 1024], dt.float32, tag="cfld", bufs=3, name="resfr")
                        drain_B(rp2, resfr, split=True)
                        wn = wk.tile([128, 1024], dt.float32r, tag="cprod", bufs=3, name="wn")
                        cmulB(wn, q12[c], resfr, conj_x=True, gp=False)
                        wu1 = drain_f32r(mm_group(statB(wn), GC_I, "wna"), "wnu")
                        wp2 = mm_group(statA(wu1), GC_I, "wnb")
                        for rb in range(2):
                            mag = wk.tile([128, 512], dt.float32, tag="mag", bufs=3, name="mag")
                            nc.scalar.activation(mag[:], wp2[rb][:], ACTF.Abs)
                            nc.sync.dma_start(out_d[c, b, rb], mag[:])

                # ---- band-pipelined driver: stage1(c)+blur(c)+CC(c), then wieners
                for c in range(NB):
                    for dl in range(DPC):
                        stage1_unit(dl, c)
                    for b in range(B):
                        blur_unit(b, c)
                    nc.gpsimd.collective_compute(
                        "AllReduce", ALU.add,
                        replica_groups=[list(range(NCORES))],
                        ins=[cc_in[c][:]], outs=[cc_out[c][:]],
                    )
                for c in range(NB):
                    for b in range(B):
                        wiener_unit(b, c)

    nc.compile()
    return nc


_PROG_CACHE = {}


def _get_program():
    if "nc" not in _PROG_CACHE:
        _PROG_CACHE["nc"] = _build_program()
    return _PROG_CACHE["nc"]


# ---------------------------------------------------------------- cached runner
def _make_runner():
    """Build the jitted SPMD callable once; reuse across kernel() calls."""
    import jax
    from jax.sharding import Mesh, PartitionSpec
    from jax.experimental.shard_map import shard_map
    import concourse.mybir as mybir
    from concourse import bass2jax

    bass2jax.install_neuronx_cc_hook()
    nc = _get_program()

    partition_name = nc.partition_id_tensor.name if nc.partition_id_tensor else None
    in_names, out_names, out_avals, zero_shapes = [], [], [], []
    for alloc in nc.m.functions[0].allocations:
        if not isinstance(alloc, mybir.MemoryLocationSet):
            continue
        if not alloc.memorylocations:
            continue
        name = alloc.memorylocations[0].name
        if alloc.kind == "ExternalInput":
            if name != partition_name:
                in_names.append(name)
        elif alloc.kind == "ExternalOutput":
            out_names.append(name)
            shape = tuple(alloc.tensor_shape)
            dtype = mybir.dt.np(alloc.dtype)
            out_avals.append(jax.core.ShapedArray(shape, dtype))
            zero_shapes.append((shape, dtype))
    n_params = len(in_names)
    n_outs = len(out_avals)
    all_in_names = list(in_names) + list(out_names)
    if partition_name is not None:
        all_in_names.append(partition_name)
    donate = tuple(range(n_params, n_params + n_outs))

    def _body(*args):
        operands = list(args)
        if partition_name is not None:
            operands.append(bass2jax.partition_id_tensor())
        outs = bass2jax._bass_exec_p.bind(
            *operands,
            out_avals=tuple(out_avals),
            in_names=tuple(all_in_names),
            out_names=tuple(out_names),
            lowering_input_output_aliases=(),
            sim_require_finite=True,
            sim_require_nnan=True,
            nc=nc,
        )
        return tuple(outs)

    devices = jax.devices()[:NCORES]
    mesh = Mesh(np.asarray(devices), ("core",))
    in_specs = (PartitionSpec("core"),) * (n_params + n_outs)
    out_specs = (PartitionSpec("core"),) * n_outs
    sharded = jax.jit(
        shard_map(_body, mesh=mesh, in_specs=in_specs, out_specs=out_specs,
                  check_rep=False),
        donate_argnums=donate, keep_unused=True)

    def run(in_maps):
        concat_in = [
            np.concatenate([np.asarray(m[name]) for m in in_maps], axis=0)
            for name in in_names
        ]
        concat_zeros = [
            np.zeros((NCORES * s[0], *s[1:]), d) for (s, d) in zero_shapes
        ]
        out_arrs = sharded(*concat_in, *concat_zeros)
        return [
            {name: np.asarray(out_arrs[i]).reshape(NCORES, *out_avals[i].shape)[c]
             for i, name in enumerate(out_names)}
            for c in range(NCORES)
        ]

    return run


def _get_runner():
    if "run" not in _PROG_CACHE:
        _PROG_CACHE["run"] = _make_runner()
    return _PROG_CACHE["run"]


# ---------------------------------------------------------------- entry point
def _build_in_maps(img, Map, H, parameter):
    # img fields (b,c) -> [128, 6144]: col = (f*2+k)*256 + x
    imgt = img.transpose(0, 3, 1, 2).reshape(B * NB, 2, 128, 256)
    imgf = np.ascontiguousarray(imgt.transpose(2, 0, 1, 3).reshape(128, B * NB * 512))
    # Map -> per-core [4, 128, DPC*512]: col = (rb*2+dl)*256 + x
    mapt = Map.transpose(3, 0, 1, 2).reshape(ND, B, 2, 128, 256)  # (d, b, rb, p, x)
    ht = np.ascontiguousarray(H.reshape(16, 16).T)
    par = parameter.reshape(1, 1)
    in_maps = []
    for core in range(NCORES):
        mp = np.zeros((B, 128, DPC * 512), np.float32)
        for dl in range(DPC):
            d = core * DPC + dl
            if d < ND:
                for rb in range(2):
                    mp[:, :, (rb * 2 + dl) * 256:(rb * 2 + dl + 1) * 256] = mapt[d, :, rb]
        in_maps.append({
            "imgf": imgf, "mapf": mp, "ht": ht, "param": par,
        })
    return in_maps


def kernel(img, Map, H, parameter):
    img = np.ascontiguousarray(np.asarray(img, np.float32))
    Map = np.ascontiguousarray(np.asarray(Map, np.float32))
    H = np.asarray(H, np.float32)
    parameter = np.asarray(parameter, np.float32)

    try:
        run = _get_runner()
    except Exception:
        run = None

    in_maps = _build_in_maps(img, Map, H, parameter)

    if run is not None:
        try:
            results = run(in_maps)
        except Exception:
            run = None
    if run is None:
        from concourse.bass_utils import run_bass_kernel_spmd
        rr = run_bass_kernel_spmd(_get_program(), in_maps,
                                  core_ids=list(range(NCORES)))
        results = rr.results

    out = np.empty((B, 256, 256, NB * ND), np.float32)
    for core in range(NCORES):
        rec = results[core]["out_recov"]            # [NB, B, 2, 128, 512]
        for dl in range(DPC):
            d = core * DPC + dl
            if d >= ND:
                continue
            for c in range(NB):
                for b in range(B):
                    blk = rec[c, b, :, :, dl * 256:(dl + 1) * 256]   # (2,128,256)
                    out[b, :, :, c * ND + d] = blk.reshape(256, 256)
    out /= out.max()
    return out



# revision 15
# speedup vs baseline: 1.1605x; 1.1605x over previous
"""Trainium2 Bass kernel for nn_Depth_CA (depth-coded-aperture Wiener pipeline).

Strategy
--------
Every fft/ifft+shift combo in the reference is a constant 256x256 complex
matrix sandwich Y = A @ X @ A.T computed on the PE array as two matmul
groups with the DATA stationary and host-precomputed constants as 512-wide
moving operands (PSUM accumulation implements the complex arithmetic).

On top of the baseline scheme, three algebraic cuts:
  * Gc == conj(Fc)/N, so psf_ifr = conj(psffr)/N^2 -- the Gc psf Gc
    sandwich is removed; the Wiener kernel K is built directly from psffr.
  * The blur and Wiener inverse transforms are real fields per depth, so
    the two depths owned by a core are PAIRED as Re/Im of one complex
    sandwich: W = Gc (X (P1 + i P2)) Gc gives both depths at once.
    Blur kernel P12 = pf_d0 + i pf_d1; Wiener kernel Q12 = Kp_d0 - i Kp_d1
    consumed via conj(Q12)*resfr (the conj is folded into the cmul).
  * blur = img (*) psf is a convolution of non-negative fields, so the
    reference's abs() is an identity and is dropped; the final global
    max-normalisation cancels mid-pipeline scaling and is done on host.

Long-lived complex fields use a [Re(rb0)|Re(rb1)|Im(rb0)|Im(rb1)] packing
so complex multiplies run as 6 [128,512] elementwise ops (2 on GpSimd).

Sharding: depths padded 15->16, 2 per core across 8 cores; per-batch
AllReduce(add) for the depth-summed `result` overlapped with blur compute.
"""
import os
import sys

for _p in ("/opt/trn_rl_repo", os.path.expanduser("~/.axon_site/_ro/trn_rl_repo")):
    if os.path.isdir(_p) and _p not in sys.path:
        sys.path.insert(0, _p)

import numpy as np

N = 256
ND, NB, B = 15, 3, 4
NDP = 16               # padded depth count
NCORES = 8
DPC = NDP // NCORES    # depths per core = 2

# ---------------------------------------------------------------- host constants
def _host_constants():
    ZI, Z0, RADII, PX = 0.05, 2.5, 0.002, 6.22e-6
    F_ = 1.0 / (1.0 / ZI + 1.0 / Z0)
    L_SEN = PX * N
    L_LEN = 2 * RADII * 2
    LAMB = np.array([460.0, 550.0, 640.0]) * 1e-9

    def deta(l_um):
        l = np.asarray(l_um, dtype=np.float64)
        return (1.5375 + 0.00829045 * l**-2 - 0.000211046 * l**-4) - 1.0

    R_ = F_ * deta(5.5e-7 * 1e6)
    FLMB = R_ / deta(LAMB * 1e6)
    ZS = np.sort(-3 * np.log(np.linspace(0.9, 11, ND)) + 8)
    DU = L_LEN / N
    u = np.arange(-L_LEN / 2, L_LEN / 2, DU)
    X_, Y_ = np.meshgrid(u, u)
    XY = X_ * X_ + Y_ * Y_
    RAD = (np.sqrt(XY) <= RADII).astype(np.float64)
    fx1 = np.fft.fftshift(np.arange(-1 / (2 * DU), 1 / (2 * DU), 1 / L_LEN))
    FX1, FY1 = np.meshgrid(fx1, fx1)
    FF = FX1 * FX1 + FY1 * FY1

    K_ = 2 * np.pi / LAMB
    COEF = (-K_ / (2 * FLMB[0]))[None, :] + K_[None, :] / (2 * ZS[:, None]) \
        + (np.pi * (L_LEN - L_SEN) / (LAMB * ZI * L_LEN))[None, :]
    PHASE1 = (COEF[:, :, None, None] * XY[None, None]).astype(np.float32)
    PHASE2 = ((np.pi * LAMB * ZI * L_LEN / L_SEN)[None, :, None, None]
              * FF[None, None]).astype(np.float32)

    W1 = RAD[None, None] * np.exp(1j * PHASE1.astype(np.float64))    # (15,3,N,N)
    W2 = np.exp(-1j * PHASE2.astype(np.float64)[0])                  # (3,N,N)

    j = np.arange(N)
    F = np.exp(-2j * np.pi * np.outer(j, j) / N)
    G = np.conj(F) / N
    P = np.zeros((N, N))
    P[j, (j + N // 2) % N] = 1.0
    A1 = F @ P
    A2 = P @ G
    Fc = P @ F @ P
    Gc = P @ G @ P
    return W1, W2, (A1, A2, Fc, Gc)


def _pack_field_B(X):
    """complex (N,N) -> float32 [128, 1024] = [Re(rb0)|Re(rb1)|Im(rb0)|Im(rb1)]."""
    out = np.empty((128, 1024), np.float32)
    for k in range(2):
        out[:, k * 256:(k + 1) * 256] = X.real[k * 128:(k + 1) * 128, :]
        out[:, 512 + k * 256:512 + (k + 1) * 256] = X.imag[k * 128:(k + 1) * 128, :]
    return out


def _pack_moving(A):
    """constant A -> float32 [2 variants, 2 k-chunks, 128, 512] moving ops."""
    AT = A.T.copy()
    out = np.empty((2, 2, 128, 512), np.float32)
    for k in range(2):
        r = AT.real[k * 128:(k + 1) * 128, :]
        i = AT.imag[k * 128:(k + 1) * 128, :]
        out[0, k, :, 0:256] = r
        out[0, k, :, 256:512] = i
        out[1, k, :, 0:256] = -i
        out[1, k, :, 256:512] = r
    return out


_CONST_CACHE = {}


def _get_device_arrays():
    """Host constants packed into the device DMA layouts (fp16 matmul path).

    Gc is pre-scaled by 1/16 so every Gc sandwich applies 1/256; the four Gc
    applications on the blur+wiener path give a uniform 1/65536 that replaces
    the reference's explicit N^2 factor in the Wiener denominator (and cancels
    in the final global max-normalisation anyway). This keeps every fp16
    intermediate comfortably inside fp16 range."""
    if "dev" not in _CONST_CACHE:
        W1, W2, mats = _host_constants()
        A1, A2, Fc, Gc = mats
        mats = (A1, A2, Fc, Gc / 16.0)
        # moving constants [128, 8192]: col = ((a*2+v)*2+k)*512 + n
        movA = np.concatenate(
            [_pack_moving(A).reshape(4, 128, 512).transpose(1, 0, 2).reshape(128, 2048)
             for A in mats], axis=1).astype(np.float16)
        # w2 [128, 3072]: col = c*1024 + layout-B
        w2p = np.concatenate([_pack_field_B(W2[c]) for c in range(NB)],
                             axis=1).astype(np.float16)
        # w1 table [48, 128, 1024] layout-B, d-major over padded depths
        w1rows = []
        for d in range(NDP):
            dd = d if d < ND else 0
            for c in range(NB):
                w1rows.append(_pack_field_B(W1[dd, c]))
        w1all = np.stack(w1rows).astype(np.float16)
        R = np.kron(np.eye(16), np.ones((1, 16))).astype(np.float32)
        _CONST_CACHE["dev"] = (np.ascontiguousarray(movA), np.ascontiguousarray(w2p),
                               np.ascontiguousarray(w1all), R)
    return _CONST_CACHE["dev"]


# ---------------------------------------------------------------- device program
_REPS = int(os.environ.get("BASS_KERNEL_REPS", "1"))

A1_I, A2_I, FC_I, GC_I = 0, 1, 2, 3


def _build_program():
    host_arrays = _get_device_arrays()
    reps = _REPS
    import concourse.bass as bass
    import concourse.bass_isa as bass_isa
    import concourse.bacc as bacc
    import concourse.mybir as mybir
    import concourse.tile as tile

    dt = mybir.dt
    ALU = mybir.AluOpType
    ACTF = mybir.ActivationFunctionType

    movA_h, w2_h, w1all_h, R_h = host_arrays

    nc = bacc.Bacc("TRN2", target_bir_lowering=False, debug=False,
                   num_devices=NCORES)

    def inline(data, name, f32r=False):
        h = nc.inline_tensor(np.ascontiguousarray(data), name=name)
        if f32r:
            mls = nc.lookup_mls(h)
            mls.dtype = dt.float32r
            h = bass.DRamTensorHandle(name, list(data.shape), dt.float32r)
        return h.ap()

    movA_d = inline(movA_h, "mova")                            # [128, 8192] fp16
    w2_d = inline(w2_h, "w2")                                  # [128, 3072] fp16
    w1all_d = inline(w1all_h, "w1all")                         # [48, 128, 1024] fp16
    r_d = inline(R_h, "rmat")                                  # [16, 256]
    onesc_d = inline(np.ones((128, 1), np.float16), "onesc")

    img_d = nc.dram_tensor("imgf", [128, 6144], dt.float16, kind="ExternalInput").ap()
    map_d = nc.dram_tensor("mapf", [B, 128, DPC * 512], dt.float16, kind="ExternalInput").ap()
    ht_d = nc.dram_tensor("ht", [16, 16], dt.float32, kind="ExternalInput").ap()
    par_d = nc.dram_tensor("param", [1, 1], dt.float32, kind="ExternalInput").ap()
    out_d = nc.dram_tensor("out_recov", [NB, B, 2, 128, 512], dt.float32, kind="ExternalOutput").ap()

    with tile.TileContext(nc) as tc:
        with (
            nc.allow_low_precision(reason="fp16 matmul path; emulated rel err 1.7e-3 vs 2e-2 gate"),
            tc.tile_pool(name="res", bufs=1) as res,
            tc.tile_pool(name="wk", bufs=2) as wk,
            tc.tile_pool(name="ps", bufs=4, space="PSUM") as ps,
            tc.tile_pool(name="dram", bufs=1, space="DRAM") as dram,
        ):
            # ---------------- resident constants (single DMAs)
            movall = res.tile([128, 8192], dt.float16, tag="movall", name="movall")
            for _a in (FC_I, A1_I, A2_I, GC_I):
                nc.sync.dma_start(movall[:, _a * 2048:(_a + 1) * 2048],
                                  movA_d[:, _a * 2048:(_a + 1) * 2048])

            def mov(a, v, k):
                o = ((a * 2 + v) * 2 + k) * 512
                return movall[:, o:o + 512]

            w2all = res.tile([128, 3072], dt.float16, tag="w2all", name="w2all")
            nc.sync.dma_start(w2all[:], w2_d[:])

            par1 = res.tile([1, 1], dt.float32, tag="par1", name="par1")
            nc.scalar.dma_start(par1[:], par_d[:])
            par128 = res.tile([128, 1], dt.float32, tag="par128", name="par128")
            nc.gpsimd.partition_broadcast(par128[:], par1[:])
            # ones vectors for matmul-based partition sum / broadcast
            ones_col = res.tile([128, 1], dt.float16, tag="ones_col", name="ones_col")
            nc.scalar.dma_start(ones_col[:], onesc_d[:])
            ones_row = res.tile([1, 128], dt.float16, tag="ones_row", name="ones_row")
            nc.vector.memset(ones_row[:], 1.0)

            # ---------------- CA = R^T @ (H @ R)  (plain fp32), both row blocks
            ht_t = res.tile([16, 16], dt.float32, tag="ht_t", name="ht_t")
            r_t = res.tile([16, 256], dt.float32, tag="r_t", name="r_t")
            nc.scalar.dma_start(ht_t[:], ht_d[:])
            nc.scalar.dma_start(r_t[:], r_d[:])
            ca_mid_ps = ps.tile([16, 256], dt.float32, tag="psB", bufs=4, name="ca_mid_ps")
            nc.tensor.matmul(ca_mid_ps[:], ht_t[:], r_t[:], start=True, stop=True)
            ca_mid = res.tile([16, 256], dt.float32, tag="ca_mid", name="ca_mid")
            nc.vector.tensor_copy(ca_mid[:], ca_mid_ps[:])
            ca_all = res.tile([128, 512], dt.float16, tag="ca_all", name="ca_all")
            for mb in range(2):
                ca_ps = ps.tile([128, 256], dt.float32, tag="psB", bufs=4, name=f"ca_ps{mb}")
                nc.tensor.matmul(ca_ps[:], r_t[:, mb * 128:(mb + 1) * 128],
                                 ca_mid[:], start=True, stop=True)
                nc.vector.tensor_copy(ca_all[:, mb * 256:(mb + 1) * 256], ca_ps[:])

            # ---------------- stationary accessors
            def statA(tiles):
                """drained MM1 pair: per-k [128,512] = [Re|Im]."""
                re = lambda k, mb: tiles[k][:, mb * 128:(mb + 1) * 128]
                im = lambda k, mb: tiles[k][:, 256 + mb * 128:256 + (mb + 1) * 128]
                return re, im

            def statB(t):
                """layout-B field [128,1024] = [Re0|Re1|Im0|Im1]."""
                re = lambda k, mb: t[:, k * 256 + mb * 128:k * 256 + (mb + 1) * 128]
                im = lambda k, mb: t[:, 512 + k * 256 + mb * 128:512 + k * 256 + (mb + 1) * 128]
                return re, im

            def statR(t):
                """real field [128,512] = [rb0|rb1]."""
                re = lambda k, mb: t[:, k * 256 + mb * 128:k * 256 + (mb + 1) * 128]
                return re, None

            MM1_NAMES = ("s1a", "s1c", "pfa", "ifa", "bla", "rfa", "wna")

            def mm_group(stat, a_idx, name):
                """PSUM[mb][128,512] = sandwich-half vs constants a_idx."""
                s_re, s_im = stat
                ptag = "psA" if name in MM1_NAMES else "psB"
                psums = []
                for mb in range(2):
                    acc = ps.tile([128, 512], dt.float32, tag=ptag, bufs=4, name=f"{name}_ps{mb}")
                    mms = []
                    for k in range(2):
                        mms.append((s_re(k, mb), mov(a_idx, 0, k)))
                        if s_im is not None:
                            mms.append((s_im(k, mb), mov(a_idx, 1, k)))
                    for i, (lhsT, rhs) in enumerate(mms):
                        nc.tensor.matmul(acc[:], lhsT, rhs,
                                         start=(i == 0), stop=(i == len(mms) - 1))
                    psums.append(acc)
                return psums

            def drain_f32r(psums, name):
                dtag, dbufs = ("drB", 6) if name in ("blu", "wnu") else ("drA", 8)
                out = [wk.tile([128, 512], dt.float16, tag=dtag, bufs=dbufs, name=f"{name}{mb}")
                       for mb in range(2)]
                nc.scalar.copy(out[0][:], psums[0][:])
                nc.vector.tensor_copy(out[1][:], psums[1][:])
                return out

            def drain_B(psums, btile, split=False):
                """drain PSUM pair into a layout-B field tile (4 copies)."""
                for rb in range(2):
                    nc.scalar.copy(btile[:, rb * 256:(rb + 1) * 256], psums[rb][:, 0:256])
                    eng = nc.vector.tensor_copy if split else (
                        lambda o, i: nc.scalar.copy(o, i))
                    eng(btile[:, 512 + rb * 256:512 + (rb + 1) * 256],
                        psums[rb][:, 256:512])

            def cmulB(out, x, y, conj_x=False, gp=True):
                """layout-B complex mult out = x*y (or conj(x)*y); 6 ops.
                gp=True offloads 2 to GpSimd (only safe for code emitted
                before the next collective on the gpsimd queue)."""
                eng2 = nc.gpsimd if gp else nc.vector
                xr, xi = x[:, 0:512], x[:, 512:1024]
                yr, yi = y[:, 0:512], y[:, 512:1024]
                t1 = wk.tile([128, 512], dt.float16, tag="cmw", bufs=7, name="cmt1")
                t2 = wk.tile([128, 512], dt.float16, tag="cmw", bufs=7, name="cmt2")
                t3 = wk.tile([128, 512], dt.float16, tag="cmw", bufs=7, name="cmt3")
                t4 = wk.tile([128, 512], dt.float16, tag="cmw", bufs=7, name="cmt4")
                nc.vector.tensor_tensor(t1[:], xr, yr, op=ALU.mult)
                eng2.tensor_tensor(t2[:], xi, yi, op=ALU.mult)
                nc.vector.tensor_tensor(t3[:], xr, yi, op=ALU.mult)
                eng2.tensor_tensor(t4[:], xi, yr, op=ALU.mult)
                if conj_x:
                    nc.vector.tensor_tensor(out[:, 0:512], t1[:], t2[:], op=ALU.add)
                    nc.vector.tensor_tensor(out[:, 512:1024], t3[:], t4[:], op=ALU.subtract)
                else:
                    nc.vector.tensor_tensor(out[:, 0:512], t1[:], t2[:], op=ALU.subtract)
                    nc.vector.tensor_tensor(out[:, 512:1024], t3[:], t4[:], op=ALU.add)

            # ---------------- resident per-core fields
            p12 = [res.tile([128, 1024], dt.float16, tag=f"p12_{c}", name=f"p12_{c}")
                   for c in range(NB)]
            q12 = [res.tile([128, 1024], dt.float16, tag=f"q12_{c}", name=f"q12_{c}")
                   for c in range(NB)]
            imgft_dr = dram.tile([B * NB, 128, 1024], dt.float16, name="imgft_dr")

            pid6 = nc.scalar.partition_id() * (DPC * NB)

            def emit_imgft(f):
                imS = wk.tile([128, 512], dt.float16, tag="imS", name="imS")
                nc.scalar.dma_start(imS[:], img_d[:, f * 512:(f + 1) * 512])
                iu1 = drain_f32r(mm_group(statR(imS), FC_I, "ifa"), "ifu")
                ip2 = mm_group(statA(iu1), FC_I, "ifb")
                imo = wk.tile([128, 1024], dt.float16, tag="cfld", bufs=3, name="imo")
                drain_B(ip2, imo)
                nc.scalar.dma_start(imgft_dr[f], imo[:])

            for _rep in range(reps):
                cc_in = [dram.tile([B, 128, 512], dt.float32, name=f"cc_in{c}_r{_rep}")
                         for c in range(NB)]
                cc_out = [dram.tile([B, 128, 512], dt.float32, name=f"cc_out{c}_r{_rep}",
                                    addr_space="Shared") for c in range(NB)]

                def stage1_unit(dl, c):
                    # imgft fields of band c, 2 per depth unit, interleaved
                    # into the psf chain's dependency gaps
                    fA, fB = (c, NB + c) if dl == 0 else (2 * NB + c, 3 * NB + c)
                    u = dl * NB + c
                    w1t = wk.tile([128, 1024], dt.float16, tag="w1t", name="w1t")
                    nc.scalar.dma_start(w1t[:], w1all_d[bass.ds(pid6 + u, 1)])
                    ph = wk.tile([128, 1024], dt.float16, tag="ph", name="ph")
                    nc.vector.tensor_tensor(ph[:, 0:512], w1t[:, 0:512], ca_all[:], op=ALU.mult)
                    nc.vector.tensor_tensor(ph[:, 512:1024], w1t[:, 512:1024], ca_all[:], op=ALU.mult)
                    u1 = drain_f32r(mm_group(statB(ph), A1_I, "s1a"), "s1u1")
                    ps2 = mm_group(statA(u1), A1_I, "s1b")
                    emit_imgft(fA)
                    # vu2 = ps2 * w2  (drain to SBUF layout-B, then wide cmul)
                    s2B = wk.tile([128, 1024], dt.float16, tag="cfld", bufs=3, name="s2B")
                    drain_B(ps2, s2B)
                    vu2 = wk.tile([128, 1024], dt.float16, tag="cprod", bufs=3, name="vu2")
                    cmulB(vu2, s2B, w2all[:, c * 1024:(c + 1) * 1024], gp=False)
                    u3 = drain_f32r(mm_group(statB(vu2), A2_I, "s1c"), "s1u3")
                    ps4 = mm_group(statA(u3), A2_I, "s1d")
                    emit_imgft(fB)
                    # psf = |vu3|^2 normalized (real field, rb-packed [128,512])
                    psfu = wk.tile([128, 512], dt.float16, tag="psfu", name="psfu")
                    for rb in range(2):
                        t1 = wk.tile([128, 256], dt.float32, tag="cms", bufs=12, name="sq1")
                        t2 = wk.tile([128, 256], dt.float32, tag="cms", bufs=12, name="sq2")
                        nc.scalar.activation(t1[:], ps4[rb][:, 0:256], ACTF.Square)
                        nc.scalar.activation(t2[:], ps4[rb][:, 256:512], ACTF.Square)
                        nc.vector.tensor_tensor(psfu[:, rb * 256:(rb + 1) * 256],
                                                t1[:], t2[:], op=ALU.add)
                    # partition sum via PE (ones^T @ psfu), then reciprocal +
                    # PE broadcast back to [128,1] -- keeps gpsimd queue free
                    srow_ps = ps.tile([1, 512], dt.float32, tag="psB", bufs=4, name="srow_ps")
                    nc.tensor.matmul(srow_ps[:], ones_col[:], psfu[:], start=True, stop=True)
                    tot1 = wk.tile([1, 1], dt.float32, tag="tot1", name="tot1")
                    nc.vector.tensor_reduce(tot1[:], srow_ps[:], axis=mybir.AxisListType.X, op=ALU.add)
                    inv1 = wk.tile([1, 1], dt.float16, tag="inv1", name="inv1")
                    nc.vector.reciprocal(inv1[:], tot1[:])
                    binv_ps = ps.tile([128, 1], dt.float32, tag="psB", bufs=4, name="binv_ps")
                    nc.tensor.matmul(binv_ps[:], ones_row[:], inv1[:], start=True, stop=True)
                    inv128 = wk.tile([128, 1], dt.float32, tag="inv128", name="inv128")
                    nc.vector.tensor_copy(inv128[:], binv_ps[:])
                    psft = wk.tile([128, 512], dt.float16, tag="psft", name="psft")
                    nc.vector.tensor_scalar_mul(psft[:], psfu[:], inv128[:])
                    # psffr = Fc psf Fc
                    pu1 = drain_f32r(mm_group(statR(psft), FC_I, "pfa"), "pfu")
                    pp2 = mm_group(statA(pu1), FC_I, "pfb")
                    # P12[c] = pf(dl=0) + i pf(dl=1); pf drained to SBUF layout-B
                    if dl == 0:
                        pfB = p12[c]
                        drain_B(pp2, pfB)
                    else:
                        pfB = wk.tile([128, 1024], dt.float32, tag="cfld", bufs=3, name="pfB")
                        drain_B(pp2, pfB)
                        nc.vector.tensor_tensor(p12[c][:, 0:512], p12[c][:, 0:512],
                                                pfB[:, 512:1024], op=ALU.subtract)
                        nc.vector.tensor_tensor(p12[c][:, 512:1024], p12[c][:, 512:1024],
                                                pfB[:, 0:512], op=ALU.add)
                    # Kp = pf / (|pf|^2 + param); Q12[c] = Kp(0) - i Kp(1)
                    # (the reference's N^2 factor is supplied by the Gc 1/16
                    #  pre-scale: four Gc applications on the blur+wiener path)
                    for rb in range(2):
                        t1 = wk.tile([128, 256], dt.float32, tag="cms", bufs=12, name="ab1")
                        t2 = wk.tile([128, 256], dt.float32, tag="cms", bufs=12, name="ab2")
                        nc.scalar.activation(t1[:], pp2[rb][:, 0:256], ACTF.Square)
                        nc.scalar.activation(t2[:], pp2[rb][:, 256:512], ACTF.Square)
                        den = wk.tile([128, 256], dt.float32, tag="cms", bufs=12, name="den")
                        nc.vector.scalar_tensor_tensor(den[:], t1[:], par128[:], t2[:],
                                                       op0=ALU.add, op1=ALU.add)
                        invp = wk.tile([128, 256], dt.float32, tag="cms", bufs=12, name="invp")
                        nc.vector.reciprocal(invp[:], den[:])
                        if dl == 0:
                            nc.vector.tensor_tensor(q12[c][:, rb * 256:(rb + 1) * 256],
                                                    pfB[:, rb * 256:(rb + 1) * 256],
                                                    invp[:], op=ALU.mult)
                            nc.vector.tensor_tensor(q12[c][:, 512 + rb * 256:512 + (rb + 1) * 256],
                                                    pfB[:, 512 + rb * 256:512 + (rb + 1) * 256],
                                                    invp[:], op=ALU.mult)
                        else:
                            kre = wk.tile([128, 256], dt.float16, tag="cms", bufs=12, name="kre")
                            kim = wk.tile([128, 256], dt.float16, tag="cms", bufs=12, name="kim")
                            nc.vector.tensor_tensor(kre[:], pfB[:, rb * 256:(rb + 1) * 256],
                                                    invp[:], op=ALU.mult)
                            nc.vector.tensor_tensor(kim[:], pfB[:, 512 + rb * 256:512 + (rb + 1) * 256],
                                                    invp[:], op=ALU.mult)
                            nc.vector.tensor_tensor(q12[c][:, rb * 256:(rb + 1) * 256],
                                                    q12[c][:, rb * 256:(rb + 1) * 256],
                                                    kim[:], op=ALU.add)
                            nc.vector.tensor_tensor(q12[c][:, 512 + rb * 256:512 + (rb + 1) * 256],
                                                    q12[c][:, 512 + rb * 256:512 + (rb + 1) * 256],
                                                    kre[:], op=ALU.subtract)

                def blur_unit(b, c):
                    mapt = wk.tile([128, DPC * 512], dt.float16, tag="mapt", bufs=2, name="mapt")
                    nc.gpsimd.dma_start(mapt[:], map_d[b])
                    f = b * NB + c
                    imf = wk.tile([128, 1024], dt.float16, tag="cfld", bufs=3, name="imf")
                    nc.scalar.dma_start(imf[:], imgft_dr[f])
                    bw = wk.tile([128, 1024], dt.float16, tag="cprod", bufs=3, name="bw")
                    cmulB(bw, imf, p12[c])
                    bu1 = drain_f32r(mm_group(statB(bw), GC_I, "bla"), "blu")
                    bp2 = mm_group(statA(bu1), GC_I, "blb")
                    # racc[rb] = sum_d map_d * W_d  (W re = d0, im = d1)
                    racc = wk.tile([128, 512], dt.float32, tag="racc", name="racc")
                    for rb in range(2):
                        t = wk.tile([128, 512], dt.float16, tag="cmw", bufs=7, name="bt")
                        nc.vector.tensor_tensor(t[:], bp2[rb][:],
                                                mapt[:, rb * 512:(rb + 1) * 512], op=ALU.mult)
                        nc.vector.tensor_tensor(racc[:, rb * 256:(rb + 1) * 256],
                                                t[:, 0:256], t[:, 256:512], op=ALU.add)
                    nc.sync.dma_start(cc_in[c][b], racc[:])

                def wiener_unit(b, c):
                        rres = wk.tile([128, 512], dt.float32, tag="rres", name="rres")
                        nc.scalar.dma_start(rres[:], cc_out[c][b])
                        res_t = wk.tile([128, 512], dt.float16, tag="res_t", name="res_t")
                        nc.vector.tensor_copy(res_t[:], rres[:])
                        ru1 = drain_f32r(mm_group(statR(res_t), FC_I, "rfa"), "rfu")
                        rp2 = mm_group(statA(ru1), FC_I, "rfb")
                        resfr = wk.tile([128, 1024], dt.float16, tag="cfld", bufs=3, name="resfr")
                        drain_B(rp2, resfr, split=True)
                        wn = wk.tile([128, 1024], dt.float16, tag="cprod", bufs=3, name="wn")
                        cmulB(wn, q12[c], resfr, conj_x=True, gp=False)
                        wu1 = drain_f32r(mm_group(statB(wn), GC_I, "wna"), "wnu")
                        wp2 = mm_group(statA(wu1), GC_I, "wnb")
                        for rb in range(2):
                            mag = wk.tile([128, 512], dt.float32, tag="mag", bufs=3, name="mag")
                            nc.scalar.activation(mag[:], wp2[rb][:], ACTF.Abs)
                            nc.sync.dma_start(out_d[c, b, rb], mag[:])

                # ---- band-pipelined driver: stage1(c)+blur(c)+CC(c), then wieners
                for c in range(NB):
                    for dl in range(DPC):
                        stage1_unit(dl, c)
                    for b in range(B):
                        blur_unit(b, c)
                    nc.gpsimd.collective_compute(
                        "AllReduce", ALU.add,
                        replica_groups=[list(range(NCORES))],
                        ins=[cc_in[c][:]], outs=[cc_out[c][:]],
                    )
                for c in range(NB):
                    for b in range(B):
                        wiener_unit(b, c)

    nc.compile()
    return nc


_PROG_CACHE = {}


def _get_program():
    if "nc" not in _PROG_CACHE:
        _PROG_CACHE["nc"] = _build_program()
    return _PROG_CACHE["nc"]


# ---------------------------------------------------------------- cached runner
def _make_runner():
    """Build the jitted SPMD callable once; reuse across kernel() calls."""
    import jax
    from jax.sharding import Mesh, PartitionSpec
    from jax.experimental.shard_map import shard_map
    import concourse.mybir as mybir
    from concourse import bass2jax

    bass2jax.install_neuronx_cc_hook()
    nc = _get_program()

    partition_name = nc.partition_id_tensor.name if nc.partition_id_tensor else None
    in_names, out_names, out_avals, zero_shapes = [], [], [], []
    for alloc in nc.m.functions[0].allocations:
        if not isinstance(alloc, mybir.MemoryLocationSet):
            continue
        if not alloc.memorylocations:
            continue
        name = alloc.memorylocations[0].name
        if alloc.kind == "ExternalInput":
            if name != partition_name:
                in_names.append(name)
        elif alloc.kind == "ExternalOutput":
            out_names.append(name)
            shape = tuple(alloc.tensor_shape)
            dtype = mybir.dt.np(alloc.dtype)
            out_avals.append(jax.core.ShapedArray(shape, dtype))
            zero_shapes.append((shape, dtype))
    n_params = len(in_names)
    n_outs = len(out_avals)
    all_in_names = list(in_names) + list(out_names)
    if partition_name is not None:
        all_in_names.append(partition_name)
    donate = tuple(range(n_params, n_params + n_outs))

    def _body(*args):
        operands = list(args)
        if partition_name is not None:
            operands.append(bass2jax.partition_id_tensor())
        outs = bass2jax._bass_exec_p.bind(
            *operands,
            out_avals=tuple(out_avals),
            in_names=tuple(all_in_names),
            out_names=tuple(out_names),
            lowering_input_output_aliases=(),
            sim_require_finite=True,
            sim_require_nnan=True,
            nc=nc,
        )
        return tuple(outs)

    devices = jax.devices()[:NCORES]
    mesh = Mesh(np.asarray(devices), ("core",))
    in_specs = (PartitionSpec("core"),) * (n_params + n_outs)
    out_specs = (PartitionSpec("core"),) * n_outs
    sharded = jax.jit(
        shard_map(_body, mesh=mesh, in_specs=in_specs, out_specs=out_specs,
                  check_rep=False),
        donate_argnums=donate, keep_unused=True)

    def run(in_maps):
        concat_in = [
            np.concatenate([np.asarray(m[name]) for m in in_maps], axis=0)
            for name in in_names
        ]
        concat_zeros = [
            np.zeros((NCORES * s[0], *s[1:]), d) for (s, d) in zero_shapes
        ]
        out_arrs = sharded(*concat_in, *concat_zeros)
        return [
            {name: np.asarray(out_arrs[i]).reshape(NCORES, *out_avals[i].shape)[c]
             for i, name in enumerate(out_names)}
            for c in range(NCORES)
        ]

    return run


def _get_runner():
    if "run" not in _PROG_CACHE:
        _PROG_CACHE["run"] = _make_runner()
    return _PROG_CACHE["run"]


# ---------------------------------------------------------------- entry point
def _build_in_maps(img, Map, H, parameter):
    # img fields (b,c) -> [128, 6144]: col = (f*2+k)*256 + x
    imgt = img.transpose(0, 3, 1, 2).reshape(B * NB, 2, 128, 256)
    imgf = np.ascontiguousarray(
        imgt.transpose(2, 0, 1, 3).reshape(128, B * NB * 512).astype(np.float16))
    # Map -> per-core [4, 128, DPC*512]: col = (rb*2+dl)*256 + x
    mapt = Map.transpose(3, 0, 1, 2).reshape(ND, B, 2, 128, 256)  # (d, b, rb, p, x)
    ht = np.ascontiguousarray(H.reshape(16, 16).T)
    par = parameter.reshape(1, 1)
    in_maps = []
    for core in range(NCORES):
        mp = np.zeros((B, 128, DPC * 512), np.float16)
        for dl in range(DPC):
            d = core * DPC + dl
            if d < ND:
                for rb in range(2):
                    mp[:, :, (rb * 2 + dl) * 256:(rb * 2 + dl + 1) * 256] = mapt[d, :, rb]
        in_maps.append({
            "imgf": imgf, "mapf": mp, "ht": ht, "param": par,
        })
    return in_maps


def kernel(img, Map, H, parameter):
    img = np.ascontiguousarray(np.asarray(img, np.float32))
    Map = np.ascontiguousarray(np.asarray(Map, np.float32))
    H = np.asarray(H, np.float32)
    parameter = np.asarray(parameter, np.float32)

    try:
        run = _get_runner()
    except Exception:
        run = None

    in_maps = _build_in_maps(img, Map, H, parameter)

    if run is not None:
        try:
            results = run(in_maps)
        except Exception:
            run = None
    if run is None:
        from concourse.bass_utils import run_bass_kernel_spmd
        rr = run_bass_kernel_spmd(_get_program(), in_maps,
                                  core_ids=list(range(NCORES)))
        results = rr.results

    out = np.empty((B, 256, 256, NB * ND), np.float32)
    for core in range(NCORES):
        rec = results[core]["out_recov"]            # [NB, B, 2, 128, 512]
        for dl in range(DPC):
            d = core * DPC + dl
            if d >= ND:
                continue
            for c in range(NB):
                for b in range(B):
                    blk = rec[c, b, :, :, dl * 256:(dl + 1) * 256]   # (2,128,256)
                    out[b, :, :, c * ND + d] = blk.reshape(256, 256)
    out /= out.max()
    return out

